# revision 44
# baseline (speedup 1.0000x reference)
"""NormalizedMutualInformationLoss Trainium2 kernel.

Strategy (data-parallel over batch, 8 batches/core on 8 cores):
  - Device loads only the even rows of x/y (the ::2 row downsample) via
    strided DMA; even-column selection happens in the on-chip compaction
    read.  The reference's 1e-4 dither and the fp16 compaction both
    perturb the NMI by < 3e-4 relative (measured 7e-5 on the real
    inputs), far inside the 2e-2 gate, so the noise stream is dropped.
  - Binning: the reference's clip -> *24 -> floor -> clip pipeline
    reduces exactly to the unary/thermometer code
      bin(v) = sum_{m=1..23} [v >= (m-12)/12].
  - Joint histogram via CDF matmul: F[n, m] = (x_n >= th_m) in bf16
    (m = 0 is the constant ones column), same for y -> G.
    S = F^T G (24x24 per batch) accumulated on the PE in fp32 PSUM
    (exact integer counts); column/row 24 of the 25x25 pair-CDF matrix
    is identically zero and synthesized on the host.  The 24x24 joint
    histogram is the 2D first difference of S, applied on the host with
    the tiny NMI log-math tail (576 bins x 64 batches).
  - e-tile layout is m-major (col = t*24*512 + m*512 + cc) so each
    threshold op writes a PACKED 2-byte slab: fp16 input + packed bf16
    output + SBUF = the DVE 4x perf mode (267 ns/threshold vs 533).
    Engine split per batch: DVE 16 thresholds, ACT 4 thresholds (Sign,
    host-recovered) + the merged fp32->fp16 compaction + the PSUM evac
    (GPSIMD cannot read PSUM), GPSIMD 3 thresholds.
  - PE: one matmul per 128-element chunk with single-strided operands
    [m:24 s=512] (the PE moving AP allows only one free dimension),
    512 matmuls/batch accumulating one 24x24 PSUM region.  Cheap
    keep-warm matmuls over the memset ones columns bridge the pipeline
    fill so the PE p-state stays ramped.
  - Evacuations are deferred two batches (psum bufs=3) so no engine's
    in-order queue stalls on a matmul tail; one output DMA at the end.
"""

import numpy as np

NB = 24          # histogram bins == CDF columns on device (m = 0..23)
B = 64           # total batch
NCORES = 8
BPC = B // NCORES  # batches per core
H = W = 512
CC = 512         # contraction chunks per batch (65536 elems / 128 partitions)
M = NB           # matmul output partitions/cols (24x24 pair-CDF)
ESTR = NB * CC   # per-tensor-half column extent of the e tile (12288)

# threshold engine split: m=1..23 real thresholds
DVE_SET = set(range(1, 17))    # 16 on DVE (is_ge, 4x mode)
ACT_SET = set(range(17, 21))   # 4 on ScalarE via Sign (host-recovered)
GP_SET = set(range(21, 24))    # 3 on GPSIMD (is_ge)
SPLIT_M = None                 # optional threshold split x/y across engines
COMPACT_ENG = "scalar"         # engine for the fp32->fp16 compaction ops
DUMMY_MM = 20                  # PE p-state warm-up matmuls over the fill
GAP_DUMMIES = 0                # keep-warm matmuls after batch 0's stream
GAP_EVERY = 0                  # keep-warm matmuls after every batch's stream

_CACHE = {}


def _ths():
    # (m-12)/12 rounded to fp32, as Python floats
    return [float(np.float32((m - 12.0) / 12.0)) for m in range(NB)]


def _split_multi_waits(nc, mybir):
    """This container's walrus accepts only one sync-wait per instruction;
    split multi-wait instructions into single-wait NoOps + the original."""
    cnt = 0
    for fn in nc.m.functions:
        for blk in fn.blocks:
            new = []
            changed = False
            for ins in blk.instructions:
                si = ins.sync_info
                if si is not None and si.on_wait and len(si.on_wait) > 1:
                    waits = list(si.on_wait)
                    for k, w in enumerate(waits[:-1]):
                        nop = mybir.InstNoOp(name=f"{ins.name}_wsplit{k}")
                        nop.engine = ins.engine
                        nop.sync_info = type(si)(on_wait=[w], on_update=[])
                        new.append(nop)
                        cnt += 1
                    ins.sync_info = type(si)(on_wait=[waits[-1]],
                                             on_update=list(si.on_update))
                    changed = True
                new.append(ins)
            if changed:
                blk.instructions = new
    return cnt


def _build_nc(dve_set=None, act_set=None, gp_set=None):
    if dve_set is None:
        dve_set, act_set, gp_set = DVE_SET, ACT_SET, GP_SET
    import concourse.bass as bass
    import concourse.mybir as mybir
    import concourse.tile as tile

    f32 = mybir.dt.float32
    f16 = mybir.dt.float16
    bf16 = mybir.dt.bfloat16
    ige = mybir.AluOpType.is_ge
    Sign = mybir.ActivationFunctionType.Sign
    Copy = mybir.ActivationFunctionType.Copy

    ths = _ths()

    nc = bass.Bass(trn_type="TRN2")
    xin = nc.dram_tensor("x", [BPC, H, W], f32, kind="ExternalInput")
    yin = nc.dram_tensor("y", [BPC, H, W], f32, kind="ExternalInput")
    sout = nc.dram_tensor("s", [M, BPC * M], f32, kind="ExternalOutput")

    # const bias APs for the ScalarE Sign ops (bias = -th)
    for m in sorted(act_set):
        v = float(-ths[m])
        if (f32, v) not in nc.const_aps.aps:
            t_ = nc.alloc_sbuf_tensor(f"const-sgn-{m}", [128, 1], f32)
            nc.gpsimd.memset(t_.ap(), v)
            nc.const_aps.aps[(f32, v)] = t_.ap()
    nc.all_engine_barrier()

    with tile.TileContext(nc) as tc:
        with (
            tc.tile_pool(name="data", bufs=4) as data_pool,
            tc.tile_pool(name="work", bufs=3) as work_pool,
            tc.tile_pool(name="exp", bufs=1) as exp_pool,
            tc.tile_pool(name="psum", bufs=3, space="PSUM") as psum_pool,
            tc.tile_pool(name="warmp", bufs=1, space="PSUM") as warm_pool,
            tc.tile_pool(name="sacc", bufs=1) as s_pool,
        ):
            s_all = s_pool.tile([M, BPC * M], f32)

            # explicit double buffer for the expansion tiles so the constant
            # ones column (m=0, both halves) is written only once per buffer
            e_tiles = []
            for i in range(2):
                e_buf = exp_pool.tile([128, 2 * ESTR], bf16, tag=f"e{i}")
                e_tiles.append(e_buf)
            for et in e_tiles:
                ev = et[:].rearrange("p (t m c) -> p m t c", t=2, m=NB)
                nc.gpsimd.memset(ev[:, 0], 1.0)
            # warm the ScalarE Sign table during the initial DMA wait
            nc.scalar.activation(e_tiles[0][:, 0:1], e_tiles[0][:, 0:1],
                                 Sign, bias=0.0, scale=1.0)

            # PE p-state warm-up: the cost model halves PE throughput for
            # 3us after any idle period.  Keep the PE continuously busy
            # across the pipeline fill with dummy matmuls over the memset
            # ones columns (finite values, result never read) so the real
            # matmul stream starts fully ramped.
            warm_p = warm_pool.tile([NB, 504], mybir.dt.float32)

            def emit_dummies(n):
                # free PE keep-warm matmuls over the memset ones columns;
                # they absorb PE idle gaps so the p-state never resets
                for _ in range(n):
                    nc.tensor.matmul(warm_p[0:NB, 0:504],
                                     e_tiles[0][:, 0:NB],
                                     e_tiles[0][:, 0:504],
                                     start=True, stop=True,
                                     skip_group_check=True)

            emit_dummies(DUMMY_MM)

            def emit_evac(pb, ppsum):
                # GPSIMD cannot read PSUM; ScalarE evacuates (tiny 24x24)
                sb = s_all[0:M, pb * M:(pb + 1) * M]
                nc.scalar.activation(sb, ppsum[:, :], Copy,
                                     bias=0.0, scale=1.0)

            def _compact(out_ap, in_ap):
                if COMPACT_ENG == "scalar":
                    nc.scalar.activation(out_ap, in_ap, Copy,
                                         bias=0.0, scale=1.0)
                elif COMPACT_ENG == "gpsimd":
                    nc.gpsimd.tensor_copy(out_ap, in_ap)
                else:
                    nc.vector.tensor_copy(out_ap, in_ap)

            def emit_load_piece(b, a_all, raw, c0, w):
                # one cc-piece [c0, c0+w) of batch b; pieces never cross the
                # j boundary (cc = j*256 + c', raw row = 4p + 2j); raw holds
                # both tensors: col = t*1024 + j*512 + rawcol
                a_r = a_all[:].rearrange("p (t j c) -> p t j c", t=2, j=2)
                raw_o = raw[:].rearrange("p (t j c) -> p t j c", t=2, j=2)
                raw_e = raw[:].rearrange("p (t j c two) -> p t j c two",
                                         t=2, j=2, two=2)[:, :, :, :, 0]
                j, cp0 = divmod(c0, W // 2)
                for half, src in enumerate((xin, yin)):
                    raw_i = src[b, 2 * j:H:4, 2 * cp0:2 * (cp0 + w)]
                    nc.sync.dma_start(
                        raw_o[:, half, j, 2 * cp0:2 * (cp0 + w)], raw_i)
                # one merged compaction over both tensors' sub-ranges
                _compact(a_r[:, :, j, cp0:cp0 + w],
                         raw_e[:, :, j, cp0:cp0 + w])

            def emit_load(b):
                # a_all holds both tensors' fp16 downsampled values:
                # x in [:, 0:512], y in [:, 512:1024]
                a_all = work_pool.tile([128, 2 * CC], f16, tag="a")
                raw = data_pool.tile([128, 4 * W], f32, tag="raw")
                raw_o = raw[:].rearrange("p (t j c) -> p t j c", t=2, j=2)
                for half, src in enumerate((xin, yin)):
                    raw_i = src[b, 0:H:2, :].rearrange("(p j) c -> p j c", j=2)
                    nc.sync.dma_start(raw_o[:, half], raw_i)
                # single merged even-column compaction + fp32 -> fp16 convert
                raw_e = raw[:].rearrange("p (u c two) -> p u c two",
                                         u=4, two=2)[:, :, :, 0]
                _compact(a_all[:], raw_e)
                return a_all

            pending = None

            def emit_thresholds(b, a_all, c0, w):
                e = e_tiles[b % 2]
                ev = e[:].rearrange("p (t m c) -> p m t c", t=2, m=NB)
                av = a_all[:].rearrange("p (t c) -> p t c", t=2)
                for m in range(1, NB):
                    if m == SPLIT_M:
                        # fine balance: x-half on DVE, y-half on GPSIMD
                        nc.vector.tensor_scalar(ev[:, m, 0, c0:c0 + w],
                                                av[:, 0, c0:c0 + w],
                                                ths[m], None, ige)
                        nc.gpsimd.tensor_scalar(ev[:, m, 1, c0:c0 + w],
                                                av[:, 1, c0:c0 + w],
                                                ths[m], None, ige)
                    elif m in act_set:
                        nc.scalar.activation(ev[:, m, :, c0:c0 + w],
                                             av[:, :, c0:c0 + w], Sign,
                                             bias=float(-ths[m]), scale=1.0)
                    elif m in gp_set:
                        nc.gpsimd.tensor_scalar(ev[:, m, :, c0:c0 + w],
                                                av[:, :, c0:c0 + w],
                                                ths[m], None, ige)
                    else:
                        nc.vector.tensor_scalar(ev[:, m, :, c0:c0 + w],
                                                av[:, :, c0:c0 + w],
                                                ths[m], None, ige)

            def emit_matmuls(b, psum, c0, w, start, stop):
                # one matmul per 128-element chunk: operands are the
                # single-strided [m:24 s=512] column sets (the PE moving AP
                # allows only one free dimension), accumulating the 24x24
                # pair-CDF in one PSUM region
                e = e_tiles[b % 2]
                ev = e[:].rearrange("p (t m c) -> p t m c", t=2, m=NB)
                for i in range(w):
                    cc = c0 + i
                    nc.tensor.matmul(
                        psum[0:NB, 0:NB],
                        ev[:, 0, :, cc],
                        ev[:, 1, :, cc],
                        start=(start and i == 0),
                        stop=(stop and i == w - 1),
                    )

            # evacs are deferred TWO batches (psum bufs=3) so Pool's
            # in-order queue has ~9us of threshold work between an evac and
            # the matmul tail it waits on
            evac_q = []

            def emit_compute(b, a_all):
                # CDF expansion, m-major: each threshold writes one packed
                # [t:2][cc:512] slab covering both tensors' halves.
                emit_thresholds(b, a_all, 0, CC)
                if len(evac_q) >= 2:
                    emit_evac(*evac_q.pop(0))
                psum = psum_pool.tile([NB, NB], mybir.dt.float32, tag="psum")
                emit_matmuls(b, psum, 0, CC, True, True)
                evac_q.append((b, psum))

            # batch 0 runs piecewise so the PE starts after a fraction of
            # the fill chain instead of the whole of it; batch 1's load is
            # emitted before batch 0's thresholds so its compaction
            # overlaps them
            PIECES = [(0, 256), (256, 256)]
            a0 = work_pool.tile([128, 2 * CC], f16, tag="a")
            raw0 = data_pool.tile([128, 4 * W], f32, tag="raw")
            for c0, w in PIECES:
                emit_load_piece(0, a0, raw0, c0, w)
            a_prev = emit_load(1)
            psum0 = psum_pool.tile([NB, NB], mybir.dt.float32, tag="psum")
            for pi, (c0, w) in enumerate(PIECES):
                emit_thresholds(0, a0, c0, w)
                emit_matmuls(0, psum0, c0, w,
                             start=(pi == 0), stop=(pi == len(PIECES) - 1))
                emit_dummies(GAP_DUMMIES)
            evac_q.append((0, psum0))

            # software pipeline: load+compact run one batch ahead of the
            # threshold/matmul stage so no engine waits on the compaction
            for b in range(2, BPC):
                a_next = emit_load(b)
                emit_compute(b - 1, a_prev)
                a_prev = a_next
            emit_compute(BPC - 1, a_prev)

            for pb, ppsum in evac_q:
                emit_evac(pb, ppsum)
            # issue the output DMA from ScalarE's own queue: its wait on the
            # evac copies is already satisfied in-order, saving a sem hop
            nc.scalar.dma_start(sout[:, :], s_all[0:M, :])

    _split_multi_waits(nc, mybir)
    return nc


def _get_nc():
    if "nc" not in _CACHE:
        _CACHE["nc"] = _build_nc()
    return _CACHE["nc"]


def _recover_steps(R):
    """R: [B, NB, NB] raw device matrices.  Sign rows/cols (ACT_SET) were
    computed as sign(a - th) = 2*step - 1; convert back to pure pair-CDF
    counts S[m,l] = #{x >= th_m, y >= th_l}."""
    N = np.float64(128 * CC)
    a = np.ones(NB, np.float64)
    bvec = np.zeros(NB, np.float64)
    for m in ACT_SET:
        a[m] = 2.0
        bvec[m] = -1.0
    CntF = (R[:, :, 0] - bvec[None, :] * N) / a[None, :]       # [B, NB]
    CntG = (R[:, 0, :] - bvec[None, :] * N) / a[None, :]       # [B, NB]
    S = (R
         - a[None, :, None] * bvec[None, None, :] * CntF[:, :, None]
         - bvec[None, :, None] * a[None, None, :] * CntG[:, None, :]
         - bvec[None, :, None] * bvec[None, None, :] * N)
    S /= a[None, :, None] * a[None, None, :]
    return S


def _nmi_tail(s_mats):
    """s_mats: [B, 25, 25] exact pair CDF counts (row/col 24 zero).
    Mirrors the reference's fp32 NMI math."""
    S = s_mats.astype(np.float32)
    J = (S[:, 0:NB, 0:NB] - S[:, 1:NB + 1, 0:NB]
         - S[:, 0:NB, 1:NB + 1] + S[:, 1:NB + 1, 1:NB + 1])
    total = (J.sum(axis=(1, 2), keepdims=True).astype(np.float32)
             + np.float32(1e-10))
    joint = (J / total).astype(np.float32)
    x_hist = joint.sum(axis=2, dtype=np.float32)
    y_hist = joint.sum(axis=1, dtype=np.float32)
    eps = np.float32(1e-5)
    joint_e = joint + eps
    xh = x_hist + eps
    yh = y_hist + eps
    log_joint = np.log(joint_e)
    log_prod = np.log(xh[:, :, None] * yh[:, None, :])
    mi = np.sum(joint_e * (log_joint - log_prod), axis=(1, 2),
                dtype=np.float32)
    hx = -np.sum(xh * np.log(xh), axis=1, dtype=np.float32)
    hy = -np.sum(yh * np.log(yh), axis=1, dtype=np.float32)
    se = hx + hy
    nmi = np.where(se < np.float32(1e-10), np.float32(0.0),
                   np.float32(2.0) * mi / se)
    nmi = np.clip(nmi, -1.0, 1.0).astype(np.float32)
    return np.float32(-np.clip(np.mean(nmi, dtype=np.float32), -1.0, 1.0))


def _run_device(x, y, trace=False):
    from concourse.bass_utils import run_bass_kernel_spmd
    nc = _get_nc()
    x = np.ascontiguousarray(np.asarray(x).reshape(B, H, W), dtype=np.float32)
    y = np.ascontiguousarray(np.asarray(y).reshape(B, H, W), dtype=np.float32)
    in_maps = [
        {"x": x[c * BPC:(c + 1) * BPC], "y": y[c * BPC:(c + 1) * BPC]}
        for c in range(NCORES)
    ]
    res = run_bass_kernel_spmd(nc, in_maps, core_ids=list(range(NCORES)),
                               trace=trace)
    # s: [24, BPC*24] per core
    R = np.zeros((B, NB, NB), dtype=np.float64)
    for c in range(NCORES):
        sc = res.results[c]["s"].astype(np.float64)
        for b in range(BPC):
            R[c * BPC + b] = sc[:, b * M:(b + 1) * M]
    S = _recover_steps(R)
    s_mats = np.zeros((B, NB + 1, NB + 1), dtype=np.float64)
    s_mats[:, 0:NB, 0:NB] = S
    return s_mats, res


def kernel(x, y):
    s_mats, _ = _run_device(x, y)
    return _nmi_tail(s_mats)


# revision 45
# speedup vs baseline: 1.0016x; 1.0016x over previous
"""NormalizedMutualInformationLoss Trainium2 kernel.

Strategy (data-parallel over batch, 8 batches/core on 8 cores):
  - Device loads only the even rows of x/y (the ::2 row downsample) via
    strided DMA; even-column selection happens in the on-chip compaction
    read.  The reference's 1e-4 dither and the fp16 compaction both
    perturb the NMI by < 3e-4 relative (measured 7e-5 on the real
    inputs), far inside the 2e-2 gate, so the noise stream is dropped.
  - Binning: the reference's clip -> *24 -> floor -> clip pipeline
    reduces exactly to the unary/thermometer code
      bin(v) = sum_{m=1..23} [v >= (m-12)/12].
  - Joint histogram via CDF matmul: F[n, m] = (x_n >= th_m) in bf16
    (m = 0 is the constant ones column), same for y -> G.
    S = F^T G (24x24 per batch) accumulated on the PE in fp32 PSUM
    (exact integer counts); column/row 24 of the 25x25 pair-CDF matrix
    is identically zero and synthesized on the host.  The 24x24 joint
    histogram is the 2D first difference of S, applied on the host with
    the tiny NMI log-math tail (576 bins x 64 batches).
  - e-tile layout is m-major (col = t*24*512 + m*512 + cc) so each
    threshold op writes a PACKED 2-byte slab: fp16 input + packed bf16
    output + SBUF = the DVE 4x perf mode (267 ns/threshold vs 533).
    Engine split per batch: DVE 16 thresholds, ACT 4 thresholds (Sign,
    host-recovered) + the merged fp32->fp16 compaction + the PSUM evac
    (GPSIMD cannot read PSUM), GPSIMD 3 thresholds.
  - PE: one matmul per 128-element chunk with single-strided operands
    [m:24 s=512] (the PE moving AP allows only one free dimension),
    512 matmuls/batch accumulating one 24x24 PSUM region.  Cheap
    keep-warm matmuls over the memset ones columns bridge the pipeline
    fill so the PE p-state stays ramped.
  - Evacuations are deferred two batches (psum bufs=3) so no engine's
    in-order queue stalls on a matmul tail; one output DMA at the end.
"""

import numpy as np

NB = 24          # histogram bins == CDF columns on device (m = 0..23)
B = 64           # total batch
NCORES = 8
BPC = B // NCORES  # batches per core
H = W = 512
CC = 512         # contraction chunks per batch (65536 elems / 128 partitions)
M = NB           # matmul output partitions/cols (24x24 pair-CDF)
ESTR = NB * CC   # per-tensor-half column extent of the e tile (12288)

# threshold engine split: m=1..23 real thresholds
DVE_SET = set(range(1, 17))    # 16 on DVE (is_ge, 4x mode)
ACT_SET = set(range(17, 21))   # 4 on ScalarE via Sign (host-recovered)
GP_SET = set(range(21, 24))    # 3 on GPSIMD (is_ge)
SPLIT_M = None                 # optional threshold split x/y across engines
COMPACT_ENG = "scalar"         # engine for the fp32->fp16 compaction ops
DUMMY_MM = 20                  # PE p-state warm-up matmuls over the fill
GAP_DUMMIES = 0                # keep-warm matmuls after batch 0's stream
GAP_EVERY = 0                  # keep-warm matmuls after every batch's stream

_CACHE = {}


def _ths():
    # (m-12)/12 rounded to fp32, as Python floats
    return [float(np.float32((m - 12.0) / 12.0)) for m in range(NB)]


def _split_multi_waits(nc, mybir):
    """This container's walrus accepts only one sync-wait per instruction;
    split multi-wait instructions into single-wait NoOps + the original."""
    cnt = 0
    for fn in nc.m.functions:
        for blk in fn.blocks:
            new = []
            changed = False
            for ins in blk.instructions:
                si = ins.sync_info
                if si is not None and si.on_wait and len(si.on_wait) > 1:
                    waits = list(si.on_wait)
                    for k, w in enumerate(waits[:-1]):
                        nop = mybir.InstNoOp(name=f"{ins.name}_wsplit{k}")
                        nop.engine = ins.engine
                        nop.sync_info = type(si)(on_wait=[w], on_update=[])
                        new.append(nop)
                        cnt += 1
                    ins.sync_info = type(si)(on_wait=[waits[-1]],
                                             on_update=list(si.on_update))
                    changed = True
                new.append(ins)
            if changed:
                blk.instructions = new
    return cnt


def _build_nc(dve_set=None, act_set=None, gp_set=None):
    if dve_set is None:
        dve_set, act_set, gp_set = DVE_SET, ACT_SET, GP_SET
    import concourse.bass as bass
    import concourse.mybir as mybir
    import concourse.tile as tile

    f32 = mybir.dt.float32
    f16 = mybir.dt.float16
    bf16 = mybir.dt.bfloat16
    ige = mybir.AluOpType.is_ge
    Sign = mybir.ActivationFunctionType.Sign
    Copy = mybir.ActivationFunctionType.Copy

    ths = _ths()

    nc = bass.Bass(trn_type="TRN2")
    xin = nc.dram_tensor("x", [BPC, H, W], f32, kind="ExternalInput")
    yin = nc.dram_tensor("y", [BPC, H, W], f32, kind="ExternalInput")
    sout = nc.dram_tensor("s", [M, BPC * M], f32, kind="ExternalOutput")

    # const bias APs for the ScalarE Sign ops (bias = -th)
    for m in sorted(act_set):
        v = float(-ths[m])
        if (f32, v) not in nc.const_aps.aps:
            t_ = nc.alloc_sbuf_tensor(f"const-sgn-{m}", [128, 1], f32)
            nc.gpsimd.memset(t_.ap(), v)
            nc.const_aps.aps[(f32, v)] = t_.ap()
    nc.all_engine_barrier()

    with tile.TileContext(nc) as tc:
        with (
            tc.tile_pool(name="data", bufs=4) as data_pool,
            tc.tile_pool(name="work", bufs=3) as work_pool,
            tc.tile_pool(name="exp", bufs=1) as exp_pool,
            tc.tile_pool(name="psum", bufs=3, space="PSUM") as psum_pool,
            tc.tile_pool(name="warmp", bufs=1, space="PSUM") as warm_pool,
            tc.tile_pool(name="sacc", bufs=1) as s_pool,
        ):
            s_all = s_pool.tile([M, BPC * M], f32)

            # explicit double buffer for the expansion tiles so the constant
            # ones column (m=0, both halves) is written only once per buffer
            e_tiles = []
            for i in range(2):
                e_buf = exp_pool.tile([128, 2 * ESTR], bf16, tag=f"e{i}")
                e_tiles.append(e_buf)
            for et in e_tiles:
                ev = et[:].rearrange("p (t m c) -> p m t c", t=2, m=NB)
                nc.gpsimd.memset(ev[:, 0], 1.0)
            # warm the ScalarE Sign table during the initial DMA wait
            nc.scalar.activation(e_tiles[0][:, 0:1], e_tiles[0][:, 0:1],
                                 Sign, bias=0.0, scale=1.0)

            # PE p-state warm-up: the cost model halves PE throughput for
            # 3us after any idle period.  Keep the PE continuously busy
            # across the pipeline fill with dummy matmuls over the memset
            # ones columns (finite values, result never read) so the real
            # matmul stream starts fully ramped.
            warm_p = warm_pool.tile([NB, 504], mybir.dt.float32)

            def emit_dummies(n):
                # free PE keep-warm matmuls over the memset ones columns;
                # they absorb PE idle gaps so the p-state never resets
                for _ in range(n):
                    nc.tensor.matmul(warm_p[0:NB, 0:504],
                                     e_tiles[0][:, 0:NB],
                                     e_tiles[0][:, 0:504],
                                     start=True, stop=True,
                                     skip_group_check=True)

            emit_dummies(DUMMY_MM)

            def emit_evac(pb, ppsum):
                # GPSIMD cannot read PSUM; ScalarE evacuates (tiny 24x24)
                sb = s_all[0:M, pb * M:(pb + 1) * M]
                nc.scalar.activation(sb, ppsum[:, :], Copy,
                                     bias=0.0, scale=1.0)

            def _compact(out_ap, in_ap):
                if COMPACT_ENG == "scalar":
                    nc.scalar.activation(out_ap, in_ap, Copy,
                                         bias=0.0, scale=1.0)
                elif COMPACT_ENG == "gpsimd":
                    nc.gpsimd.tensor_copy(out_ap, in_ap)
                else:
                    nc.vector.tensor_copy(out_ap, in_ap)

            def emit_load_piece(b, a_all, raw, c0, w):
                # one cc-piece [c0, c0+w) of batch b; pieces never cross the
                # j boundary (cc = j*256 + c', raw row = 4p + 2j); raw holds
                # both tensors: col = t*1024 + j*512 + rawcol
                a_r = a_all[:].rearrange("p (t j c) -> p t j c", t=2, j=2)
                raw_o = raw[:].rearrange("p (t j c) -> p t j c", t=2, j=2)
                raw_e = raw[:].rearrange("p (t j c two) -> p t j c two",
                                         t=2, j=2, two=2)[:, :, :, :, 0]
                j, cp0 = divmod(c0, W // 2)
                for half, src in enumerate((xin, yin)):
                    raw_i = src[b, 2 * j:H:4, 2 * cp0:2 * (cp0 + w)]
                    nc.sync.dma_start(
                        raw_o[:, half, j, 2 * cp0:2 * (cp0 + w)], raw_i)
                # one merged compaction over both tensors' sub-ranges
                _compact(a_r[:, :, j, cp0:cp0 + w],
                         raw_e[:, :, j, cp0:cp0 + w])

            def emit_load(b):
                # a_all holds both tensors' fp16 downsampled values:
                # x in [:, 0:512], y in [:, 512:1024]
                a_all = work_pool.tile([128, 2 * CC], f16, tag="a")
                raw = data_pool.tile([128, 4 * W], f32, tag="raw")
                raw_o = raw[:].rearrange("p (t j c) -> p t j c", t=2, j=2)
                for half, src in enumerate((xin, yin)):
                    raw_i = src[b, 0:H:2, :].rearrange("(p j) c -> p j c", j=2)
                    nc.sync.dma_start(raw_o[:, half], raw_i)
                # single merged even-column compaction + fp32 -> fp16 convert
                raw_e = raw[:].rearrange("p (u c two) -> p u c two",
                                         u=4, two=2)[:, :, :, 0]
                _compact(a_all[:], raw_e)
                return a_all

            pending = None

            def emit_thresholds(b, a_all, c0, w):
                e = e_tiles[b % 2]
                ev = e[:].rearrange("p (t m c) -> p m t c", t=2, m=NB)
                av = a_all[:].rearrange("p (t c) -> p t c", t=2)
                for m in range(1, NB):
                    if m == SPLIT_M:
                        # fine balance: x-half on DVE, y-half on GPSIMD
                        nc.vector.tensor_scalar(ev[:, m, 0, c0:c0 + w],
                                                av[:, 0, c0:c0 + w],
                                                ths[m], None, ige)
                        nc.gpsimd.tensor_scalar(ev[:, m, 1, c0:c0 + w],
                                                av[:, 1, c0:c0 + w],
                                                ths[m], None, ige)
                    elif m in act_set:
                        nc.scalar.activation(ev[:, m, :, c0:c0 + w],
                                             av[:, :, c0:c0 + w], Sign,
                                             bias=float(-ths[m]), scale=1.0)
                    elif m in gp_set:
                        nc.gpsimd.tensor_scalar(ev[:, m, :, c0:c0 + w],
                                                av[:, :, c0:c0 + w],
                                                ths[m], None, ige)
                    else:
                        nc.vector.tensor_scalar(ev[:, m, :, c0:c0 + w],
                                                av[:, :, c0:c0 + w],
                                                ths[m], None, ige)

            def emit_matmuls(b, psum, c0, w, start, stop):
                # one matmul per 128-element chunk: operands are the
                # single-strided [m:24 s=512] column sets (the PE moving AP
                # allows only one free dimension), accumulating the 24x24
                # pair-CDF in one PSUM region
                e = e_tiles[b % 2]
                ev = e[:].rearrange("p (t m c) -> p t m c", t=2, m=NB)
                for i in range(w):
                    cc = c0 + i
                    nc.tensor.matmul(
                        psum[0:NB, 0:NB],
                        ev[:, 0, :, cc],
                        ev[:, 1, :, cc],
                        start=(start and i == 0),
                        stop=(stop and i == w - 1),
                    )

            # evacs are deferred TWO batches (psum bufs=3) so Pool's
            # in-order queue has ~9us of threshold work between an evac and
            # the matmul tail it waits on
            evac_q = []

            def emit_compute(b, a_all):
                # CDF expansion, m-major: each threshold writes one packed
                # [t:2][cc:512] slab covering both tensors' halves.
                emit_thresholds(b, a_all, 0, CC)
                if len(evac_q) >= 2:
                    emit_evac(*evac_q.pop(0))
                psum = psum_pool.tile([NB, NB], mybir.dt.float32, tag="psum")
                emit_matmuls(b, psum, 0, CC, True, True)
                evac_q.append((b, psum))

            # batch 0 runs piecewise so the PE starts after a fraction of
            # the fill chain instead of the whole of it; batch 1's load is
            # emitted before batch 0's thresholds so its compaction
            # overlaps them
            PIECES = [(0, 256), (256, 256)]
            a0 = work_pool.tile([128, 2 * CC], f16, tag="a")
            raw0 = data_pool.tile([128, 4 * W], f32, tag="raw")
            for c0, w in PIECES:
                emit_load_piece(0, a0, raw0, c0, w)
            a_prev = emit_load(1)
            psum0 = psum_pool.tile([NB, NB], mybir.dt.float32, tag="psum")
            for pi, (c0, w) in enumerate(PIECES):
                emit_thresholds(0, a0, c0, w)
                emit_matmuls(0, psum0, c0, w,
                             start=(pi == 0), stop=(pi == len(PIECES) - 1))
                emit_dummies(GAP_DUMMIES)
            evac_q.append((0, psum0))

            # software pipeline: load+compact run one batch ahead of the
            # threshold/matmul stage so no engine waits on the compaction
            for b in range(2, BPC):
                a_next = emit_load(b)
                emit_compute(b - 1, a_prev)
                a_prev = a_next
            emit_compute(BPC - 1, a_prev)

            for pb, ppsum in evac_q:
                emit_evac(pb, ppsum)
            nc.sync.dma_start(sout[:, :], s_all[0:M, :])

    _split_multi_waits(nc, mybir)
    return nc


def _get_nc():
    if "nc" not in _CACHE:
        _CACHE["nc"] = _build_nc()
    return _CACHE["nc"]


def _recover_steps(R):
    """R: [B, NB, NB] raw device matrices.  Sign rows/cols (ACT_SET) were
    computed as sign(a - th) = 2*step - 1; convert back to pure pair-CDF
    counts S[m,l] = #{x >= th_m, y >= th_l}."""
    N = np.float64(128 * CC)
    a = np.ones(NB, np.float64)
    bvec = np.zeros(NB, np.float64)
    for m in ACT_SET:
        a[m] = 2.0
        bvec[m] = -1.0
    CntF = (R[:, :, 0] - bvec[None, :] * N) / a[None, :]       # [B, NB]
    CntG = (R[:, 0, :] - bvec[None, :] * N) / a[None, :]       # [B, NB]
    S = (R
         - a[None, :, None] * bvec[None, None, :] * CntF[:, :, None]
         - bvec[None, :, None] * a[None, None, :] * CntG[:, None, :]
         - bvec[None, :, None] * bvec[None, None, :] * N)
    S /= a[None, :, None] * a[None, None, :]
    return S


def _nmi_tail(s_mats):
    """s_mats: [B, 25, 25] exact pair CDF counts (row/col 24 zero).
    Mirrors the reference's fp32 NMI math."""
    S = s_mats.astype(np.float32)
    J = (S[:, 0:NB, 0:NB] - S[:, 1:NB + 1, 0:NB]
         - S[:, 0:NB, 1:NB + 1] + S[:, 1:NB + 1, 1:NB + 1])
    total = (J.sum(axis=(1, 2), keepdims=True).astype(np.float32)
             + np.float32(1e-10))
    joint = (J / total).astype(np.float32)
    x_hist = joint.sum(axis=2, dtype=np.float32)
    y_hist = joint.sum(axis=1, dtype=np.float32)
    eps = np.float32(1e-5)
    joint_e = joint + eps
    xh = x_hist + eps
    yh = y_hist + eps
    log_joint = np.log(joint_e)
    log_prod = np.log(xh[:, :, None] * yh[:, None, :])
    mi = np.sum(joint_e * (log_joint - log_prod), axis=(1, 2),
                dtype=np.float32)
    hx = -np.sum(xh * np.log(xh), axis=1, dtype=np.float32)
    hy = -np.sum(yh * np.log(yh), axis=1, dtype=np.float32)
    se = hx + hy
    nmi = np.where(se < np.float32(1e-10), np.float32(0.0),
                   np.float32(2.0) * mi / se)
    nmi = np.clip(nmi, -1.0, 1.0).astype(np.float32)
    return np.float32(-np.clip(np.mean(nmi, dtype=np.float32), -1.0, 1.0))


def _run_device(x, y, trace=False):
    from concourse.bass_utils import run_bass_kernel_spmd
    nc = _get_nc()
    x = np.ascontiguousarray(np.asarray(x).reshape(B, H, W), dtype=np.float32)
    y = np.ascontiguousarray(np.asarray(y).reshape(B, H, W), dtype=np.float32)
    in_maps = [
        {"x": x[c * BPC:(c + 1) * BPC], "y": y[c * BPC:(c + 1) * BPC]}
        for c in range(NCORES)
    ]
    res = run_bass_kernel_spmd(nc, in_maps, core_ids=list(range(NCORES)),
                               trace=trace)
    # s: [24, BPC*24] per core
    R = np.zeros((B, NB, NB), dtype=np.float64)
    for c in range(NCORES):
        sc = res.results[c]["s"].astype(np.float64)
        for b in range(BPC):
            R[c * BPC + b] = sc[:, b * M:(b + 1) * M]
    S = _recover_steps(R)
    s_mats = np.zeros((B, NB + 1, NB + 1), dtype=np.float64)
    s_mats[:, 0:NB, 0:NB] = S
    return s_mats, res


def kernel(x, y):
    s_mats, _ = _run_device(x, y)
    return _nmi_tail(s_mats)


# revision 47
# speedup vs baseline: 1.0071x; 1.0055x over previous
"""NormalizedMutualInformationLoss Trainium2 kernel.

Strategy (data-parallel over batch, 8 batches/core on 8 cores):
  - Device loads only the even rows of x/y (the ::2 row downsample) via
    strided DMA; even-column selection happens in the on-chip compaction
    read.  The reference's 1e-4 dither and the fp16 compaction both
    perturb the NMI by < 3e-4 relative (measured 7e-5 on the real
    inputs), far inside the 2e-2 gate, so the noise stream is dropped.
  - Binning: the reference's clip -> *24 -> floor -> clip pipeline
    reduces exactly to the unary/thermometer code
      bin(v) = sum_{m=1..23} [v >= (m-12)/12].
  - Joint histogram via CDF matmul: F[n, m] = (x_n >= th_m) in bf16
    (m = 0 is the constant ones column), same for y -> G.
    S = F^T G (24x24 per batch) accumulated on the PE in fp32 PSUM
    (exact integer counts); column/row 24 of the 25x25 pair-CDF matrix
    is identically zero and synthesized on the host.  The 24x24 joint
    histogram is the 2D first difference of S, applied on the host with
    the tiny NMI log-math tail (576 bins x 64 batches).
  - e-tile layout is m-major (col = t*24*512 + m*512 + cc) so each
    threshold op writes a PACKED 2-byte slab: fp16 input + packed bf16
    output + SBUF = the DVE 4x perf mode (267 ns/threshold vs 533).
    Engine split per batch: DVE 16 thresholds, ACT 4 thresholds (Sign,
    host-recovered) + the merged fp32->fp16 compaction + the PSUM evac
    (GPSIMD cannot read PSUM), GPSIMD 3 thresholds.
  - PE: one matmul per 128-element chunk with single-strided operands
    [m:24 s=512] (the PE moving AP allows only one free dimension),
    512 matmuls/batch accumulating one 24x24 PSUM region.  Cheap
    keep-warm matmuls over the memset ones columns bridge the pipeline
    fill so the PE p-state stays ramped.
  - Evacuations are deferred two batches (psum bufs=3) so no engine's
    in-order queue stalls on a matmul tail; one output DMA at the end.
"""

import numpy as np

NB = 24          # histogram bins == CDF columns on device (m = 0..23)
B = 64           # total batch
NCORES = 8
BPC = B // NCORES  # batches per core
H = W = 512
CC = 512         # contraction chunks per batch (65536 elems / 128 partitions)
M = NB           # matmul output partitions/cols (24x24 pair-CDF)
ESTR = NB * CC   # per-tensor-half column extent of the e tile (12288)

# threshold engine split: m=1..23 real thresholds
DVE_SET = set(range(1, 16))    # 15 full thresholds on DVE (is_ge, 4x mode)
ACT_SET = set(range(16, 20))   # 4 on ScalarE via Sign (host-recovered)
GP_SET = set(range(20, 23))    # 3 on GPSIMD (is_ge)
SPLIT_M = 23                   # x-half on DVE, y-half on GPSIMD (both step)
COMPACT_ENG = "scalar"         # engine for the fp32->fp16 compaction ops
DUMMY_MM = 20                  # PE p-state warm-up matmuls over the fill
GAP_DUMMIES = 0                # keep-warm matmuls after batch 0's stream
GAP_EVERY = 0                  # keep-warm matmuls after every batch's stream

_CACHE = {}


def _ths():
    # (m-12)/12 rounded to fp32, as Python floats
    return [float(np.float32((m - 12.0) / 12.0)) for m in range(NB)]


def _split_multi_waits(nc, mybir):
    """This container's walrus accepts only one sync-wait per instruction;
    split multi-wait instructions into single-wait NoOps + the original."""
    cnt = 0
    for fn in nc.m.functions:
        for blk in fn.blocks:
            new = []
            changed = False
            for ins in blk.instructions:
                si = ins.sync_info
                if si is not None and si.on_wait and len(si.on_wait) > 1:
                    waits = list(si.on_wait)
                    for k, w in enumerate(waits[:-1]):
                        nop = mybir.InstNoOp(name=f"{ins.name}_wsplit{k}")
                        nop.engine = ins.engine
                        nop.sync_info = type(si)(on_wait=[w], on_update=[])
                        new.append(nop)
                        cnt += 1
                    ins.sync_info = type(si)(on_wait=[waits[-1]],
                                             on_update=list(si.on_update))
                    changed = True
                new.append(ins)
            if changed:
                blk.instructions = new
    return cnt


def _build_nc(dve_set=None, act_set=None, gp_set=None):
    if dve_set is None:
        dve_set, act_set, gp_set = DVE_SET, ACT_SET, GP_SET
    import concourse.bass as bass
    import concourse.mybir as mybir
    import concourse.tile as tile

    f32 = mybir.dt.float32
    f16 = mybir.dt.float16
    bf16 = mybir.dt.bfloat16
    ige = mybir.AluOpType.is_ge
    Sign = mybir.ActivationFunctionType.Sign
    Copy = mybir.ActivationFunctionType.Copy

    ths = _ths()

    nc = bass.Bass(trn_type="TRN2")
    xin = nc.dram_tensor("x", [BPC, H, W], f32, kind="ExternalInput")
    yin = nc.dram_tensor("y", [BPC, H, W], f32, kind="ExternalInput")
    sout = nc.dram_tensor("s", [M, (BPC + 1) * M], f32,
                          kind="ExternalOutput")

    # const bias APs for the ScalarE Sign ops (bias = -th)
    for m in sorted(act_set):
        v = float(-ths[m])
        if (f32, v) not in nc.const_aps.aps:
            t_ = nc.alloc_sbuf_tensor(f"const-sgn-{m}", [128, 1], f32)
            nc.gpsimd.memset(t_.ap(), v)
            nc.const_aps.aps[(f32, v)] = t_.ap()
    nc.all_engine_barrier()

    with tile.TileContext(nc) as tc:
        with (
            tc.tile_pool(name="data", bufs=4) as data_pool,
            tc.tile_pool(name="work", bufs=3) as work_pool,
            tc.tile_pool(name="exp", bufs=1) as exp_pool,
            tc.tile_pool(name="psum", bufs=3, space="PSUM") as psum_pool,
            tc.tile_pool(name="warmp", bufs=1, space="PSUM") as warm_pool,
            tc.tile_pool(name="sacc", bufs=1) as s_pool,
        ):
            s_all = s_pool.tile([M, (BPC + 1) * M], f32)

            # explicit double buffer for the expansion tiles so the constant
            # ones column (m=0, both halves) is written only once per buffer
            e_tiles = []
            for i in range(2):
                e_buf = exp_pool.tile([128, 2 * ESTR], bf16, tag=f"e{i}")
                e_tiles.append(e_buf)
            for et in e_tiles:
                ev = et[:].rearrange("p (t m c) -> p m t c", t=2, m=NB)
                nc.gpsimd.memset(ev[:, 0], 1.0)
            # warm the ScalarE Sign table during the initial DMA wait
            nc.scalar.activation(e_tiles[0][:, 0:1], e_tiles[0][:, 0:1],
                                 Sign, bias=0.0, scale=1.0)

            # PE p-state warm-up: the cost model halves PE throughput for
            # 3us after any idle period.  Keep the PE continuously busy
            # across the pipeline fill with dummy matmuls over the memset
            # ones columns (finite values, result never read) so the real
            # matmul stream starts fully ramped.
            warm_p = warm_pool.tile([NB, 504], mybir.dt.float32)

            def emit_dummies(n):
                # free PE keep-warm matmuls over the memset ones columns;
                # they absorb PE idle gaps so the p-state never resets
                for _ in range(n):
                    nc.tensor.matmul(warm_p[0:NB, 0:504],
                                     e_tiles[0][:, 0:NB],
                                     e_tiles[0][:, 0:504],
                                     start=True, stop=True,
                                     skip_group_check=True)

            emit_dummies(DUMMY_MM)

            def emit_evac(pb, ppsum):
                # GPSIMD cannot read PSUM; ScalarE evacuates (tiny 24x24)
                sb = s_all[0:M, pb * M:(pb + 1) * M]
                nc.scalar.activation(sb, ppsum[:, :], Copy,
                                     bias=0.0, scale=1.0)

            def _compact(out_ap, in_ap):
                if COMPACT_ENG == "scalar":
                    nc.scalar.activation(out_ap, in_ap, Copy,
                                         bias=0.0, scale=1.0)
                elif COMPACT_ENG == "gpsimd":
                    nc.gpsimd.tensor_copy(out_ap, in_ap)
                else:
                    nc.vector.tensor_copy(out_ap, in_ap)

            def emit_load_piece(b, a_all, raw, c0, w):
                # one cc-piece [c0, c0+w) of batch b; pieces never cross the
                # j boundary (cc = j*256 + c', raw row = 4p + 2j); raw holds
                # both tensors: col = t*1024 + j*512 + rawcol
                a_r = a_all[:].rearrange("p (t j c) -> p t j c", t=2, j=2)
                raw_o = raw[:].rearrange("p (t j c) -> p t j c", t=2, j=2)
                raw_e = raw[:].rearrange("p (t j c two) -> p t j c two",
                                         t=2, j=2, two=2)[:, :, :, :, 0]
                j, cp0 = divmod(c0, W // 2)
                for half, src in enumerate((xin, yin)):
                    raw_i = src[b, 2 * j:H:4, 2 * cp0:2 * (cp0 + w)]
                    nc.sync.dma_start(
                        raw_o[:, half, j, 2 * cp0:2 * (cp0 + w)], raw_i)
                # one merged compaction over both tensors' sub-ranges
                _compact(a_r[:, :, j, cp0:cp0 + w],
                         raw_e[:, :, j, cp0:cp0 + w])

            def emit_load(b):
                # a_all holds both tensors' fp16 downsampled values:
                # x in [:, 0:512], y in [:, 512:1024]
                a_all = work_pool.tile([128, 2 * CC], f16, tag="a")
                raw = data_pool.tile([128, 4 * W], f32, tag="raw")
                raw_o = raw[:].rearrange("p (t j c) -> p t j c", t=2, j=2)
                for half, src in enumerate((xin, yin)):
                    raw_i = src[b, 0:H:2, :].rearrange("(p j) c -> p j c", j=2)
                    nc.sync.dma_start(raw_o[:, half], raw_i)
                # single merged even-column compaction + fp32 -> fp16 convert
                raw_e = raw[:].rearrange("p (u c two) -> p u c two",
                                         u=4, two=2)[:, :, :, 0]
                _compact(a_all[:], raw_e)
                return a_all

            pending = None

            def emit_thresholds(b, a_all, c0, w):
                e = e_tiles[b % 2]
                ev = e[:].rearrange("p (t m c) -> p m t c", t=2, m=NB)
                av = a_all[:].rearrange("p (t c) -> p t c", t=2)
                for m in range(1, NB):
                    if m == SPLIT_M:
                        # fine balance: x-half on DVE, y-half on GPSIMD
                        nc.vector.tensor_scalar(ev[:, m, 0, c0:c0 + w],
                                                av[:, 0, c0:c0 + w],
                                                ths[m], None, ige)
                        nc.gpsimd.tensor_scalar(ev[:, m, 1, c0:c0 + w],
                                                av[:, 1, c0:c0 + w],
                                                ths[m], None, ige)
                    elif m in act_set:
                        nc.scalar.activation(ev[:, m, :, c0:c0 + w],
                                             av[:, :, c0:c0 + w], Sign,
                                             bias=float(-ths[m]), scale=1.0)
                    elif m in gp_set:
                        nc.gpsimd.tensor_scalar(ev[:, m, :, c0:c0 + w],
                                                av[:, :, c0:c0 + w],
                                                ths[m], None, ige)
                    else:
                        nc.vector.tensor_scalar(ev[:, m, :, c0:c0 + w],
                                                av[:, :, c0:c0 + w],
                                                ths[m], None, ige)

            def emit_matmuls(b, psum, c0, w, start, stop):
                # one matmul per 128-element chunk: operands are the
                # single-strided [m:24 s=512] column sets (the PE moving AP
                # allows only one free dimension), accumulating the 24x24
                # pair-CDF in one PSUM region
                e = e_tiles[b % 2]
                ev = e[:].rearrange("p (t m c) -> p t m c", t=2, m=NB)
                for i in range(w):
                    cc = c0 + i
                    nc.tensor.matmul(
                        psum[0:NB, 0:NB],
                        ev[:, 0, :, cc],
                        ev[:, 1, :, cc],
                        start=(start and i == 0),
                        stop=(stop and i == w - 1),
                    )

            # evacs are deferred TWO batches (psum bufs=3) so Pool's
            # in-order queue has ~9us of threshold work between an evac and
            # the matmul tail it waits on
            evac_q = []

            def emit_compute(b, a_all):
                # CDF expansion, m-major: each threshold writes one packed
                # [t:2][cc:512] slab covering both tensors' halves.
                if b < BPC - 1:
                    emit_thresholds(b, a_all, 0, CC)
                    if len(evac_q) >= 2:
                        emit_evac(*evac_q.pop(0))
                    psum = psum_pool.tile([NB, NB], mybir.dt.float32,
                                          tag="psum")
                    emit_matmuls(b, psum, 0, CC, True, True)
                    evac_q.append((b, psum))
                else:
                    # final batch: two SEPARATE complete accumulation groups
                    # (A: chunks 0..383, B: 384..511, host sums the blocks)
                    # so the matmul tail overlaps the threshold stage
                    emit_thresholds(b, a_all, 0, 384)
                    if len(evac_q) >= 2:
                        emit_evac(*evac_q.pop(0))
                    psumA = psum_pool.tile([NB, NB], mybir.dt.float32,
                                           tag="psum")
                    emit_matmuls(b, psumA, 0, 384, True, True)
                    emit_thresholds(b, a_all, 384, 128)
                    psumB = psum_pool.tile([NB, NB], mybir.dt.float32,
                                           tag="psum")
                    emit_matmuls(b, psumB, 384, 128, True, True)
                    evac_q.append((b, psumA))
                    evac_q.append((BPC, psumB))

            # batch 0 runs piecewise so the PE starts after a fraction of
            # the fill chain instead of the whole of it; batch 1's load is
            # emitted before batch 0's thresholds so its compaction
            # overlaps them
            PIECES = [(0, 256), (256, 256)]
            a0 = work_pool.tile([128, 2 * CC], f16, tag="a")
            raw0 = data_pool.tile([128, 4 * W], f32, tag="raw")
            for c0, w in PIECES:
                emit_load_piece(0, a0, raw0, c0, w)
            a_prev = emit_load(1)
            psum0 = psum_pool.tile([NB, NB], mybir.dt.float32, tag="psum")
            for pi, (c0, w) in enumerate(PIECES):
                emit_thresholds(0, a0, c0, w)
                emit_matmuls(0, psum0, c0, w,
                             start=(pi == 0), stop=(pi == len(PIECES) - 1))
                emit_dummies(GAP_DUMMIES)
            evac_q.append((0, psum0))

            # software pipeline: load+compact run one batch ahead of the
            # threshold/matmul stage so no engine waits on the compaction
            for b in range(2, BPC):
                a_next = emit_load(b)
                emit_compute(b - 1, a_prev)
                a_prev = a_next
            emit_compute(BPC - 1, a_prev)

            for pb, ppsum in evac_q:
                emit_evac(pb, ppsum)
            nc.sync.dma_start(sout[:, :], s_all[0:M, :])

    _split_multi_waits(nc, mybir)
    return nc


def _get_nc():
    if "nc" not in _CACHE:
        _CACHE["nc"] = _build_nc()
    return _CACHE["nc"]


def _recover_steps(R):
    """R: [B, NB, NB] raw device matrices.  Sign rows/cols (ACT_SET) were
    computed as sign(a - th) = 2*step - 1; convert back to pure pair-CDF
    counts S[m,l] = #{x >= th_m, y >= th_l}."""
    N = np.float64(128 * CC)
    a = np.ones(NB, np.float64)
    bvec = np.zeros(NB, np.float64)
    for m in ACT_SET:
        a[m] = 2.0
        bvec[m] = -1.0
    CntF = (R[:, :, 0] - bvec[None, :] * N) / a[None, :]       # [B, NB]
    CntG = (R[:, 0, :] - bvec[None, :] * N) / a[None, :]       # [B, NB]
    S = (R
         - a[None, :, None] * bvec[None, None, :] * CntF[:, :, None]
         - bvec[None, :, None] * a[None, None, :] * CntG[:, None, :]
         - bvec[None, :, None] * bvec[None, None, :] * N)
    S /= a[None, :, None] * a[None, None, :]
    return S


def _nmi_tail(s_mats):
    """s_mats: [B, 25, 25] exact pair CDF counts (row/col 24 zero).
    Mirrors the reference's fp32 NMI math."""
    S = s_mats.astype(np.float32)
    J = (S[:, 0:NB, 0:NB] - S[:, 1:NB + 1, 0:NB]
         - S[:, 0:NB, 1:NB + 1] + S[:, 1:NB + 1, 1:NB + 1])
    total = (J.sum(axis=(1, 2), keepdims=True).astype(np.float32)
             + np.float32(1e-10))
    joint = (J / total).astype(np.float32)
    x_hist = joint.sum(axis=2, dtype=np.float32)
    y_hist = joint.sum(axis=1, dtype=np.float32)
    eps = np.float32(1e-5)
    joint_e = joint + eps
    xh = x_hist + eps
    yh = y_hist + eps
    log_joint = np.log(joint_e)
    log_prod = np.log(xh[:, :, None] * yh[:, None, :])
    mi = np.sum(joint_e * (log_joint - log_prod), axis=(1, 2),
                dtype=np.float32)
    hx = -np.sum(xh * np.log(xh), axis=1, dtype=np.float32)
    hy = -np.sum(yh * np.log(yh), axis=1, dtype=np.float32)
    se = hx + hy
    nmi = np.where(se < np.float32(1e-10), np.float32(0.0),
                   np.float32(2.0) * mi / se)
    nmi = np.clip(nmi, -1.0, 1.0).astype(np.float32)
    return np.float32(-np.clip(np.mean(nmi, dtype=np.float32), -1.0, 1.0))


def _run_device(x, y, trace=False):
    from concourse.bass_utils import run_bass_kernel_spmd
    nc = _get_nc()
    x = np.ascontiguousarray(np.asarray(x).reshape(B, H, W), dtype=np.float32)
    y = np.ascontiguousarray(np.asarray(y).reshape(B, H, W), dtype=np.float32)
    in_maps = [
        {"x": x[c * BPC:(c + 1) * BPC], "y": y[c * BPC:(c + 1) * BPC]}
        for c in range(NCORES)
    ]
    res = run_bass_kernel_spmd(nc, in_maps, core_ids=list(range(NCORES)),
                               trace=trace)
    # s: [24, (BPC+1)*24] per core; slot BPC holds the second half of
    # the last batch's accumulation
    R = np.zeros((B, NB, NB), dtype=np.float64)
    for c in range(NCORES):
        sc = res.results[c]["s"].astype(np.float64)
        for b in range(BPC):
            R[c * BPC + b] = sc[:, b * M:(b + 1) * M]
        R[c * BPC + BPC - 1] += sc[:, BPC * M:(BPC + 1) * M]
    S = _recover_steps(R)
    s_mats = np.zeros((B, NB + 1, NB + 1), dtype=np.float64)
    s_mats[:, 0:NB, 0:NB] = S
    return s_mats, res


def kernel(x, y):
    s_mats, _ = _run_device(x, y)
    return _nmi_tail(s_mats)
